# revision 9
# baseline (speedup 1.0000x reference)
"""GraphSAGE (2x SAGEConv + global mean pool + FC + sigmoid) on 8 TRN2 NeuronCores.

Strategy
--------
The SAGEConv projection commutes with mean aggregation:
    h = relu([x | mean_nbr(x)] @ W1) = relu(x @ W1_top + mean_nbr(x @ W1_bot))
so we project to DIM=10 first and only ever gather 10(->16 padded)-value rows.

Sharding: nodes are globally sorted by in-degree (desc) and dealt round-robin
to the 8 cores, so every core has an identical per-block degree profile ->
one SPMD program with compile-time-uniform gather counts per 128-node block.

Gathers use the DMAGatherAnt ucode (batched dma_gather, 256B elements = 4
nodes x 16 f32). Each 128-dst block gathers its K*128 edge slots in a few
calls (<= CAP indices each); a DVE multiply against a host-built f32 mask
performs 4-way lane-select + edge-pad masking + the 1/deg mean in one op,
restricted to the DIM=10 live features. Tables are exchanged with AllGather
collectives; pooling uses the same machinery with 1/graph-size masks.
"""

import numpy as np

N = 100_000
B = 1000
F_IN = 128
DIM = 10
NCORES = 8
PERC = 12544            # nodes per core (98 blocks of 128); 12500 real + 44 dummy
NB = PERC // 128        # 98
NTOT = PERC * NCORES    # 100352
LANES = 4               # nodes per 256B table row (f32)
R4 = NTOT // LANES      # packed table rows
F16 = 16                # padded feature width
GPC = B // NCORES       # 125 graphs per core
SBLK = 7                # blocks per idx/mask streaming superblock
CAP = 1024              # max indices per dma_gather call

_CACHE: dict = {}


# ----------------------------------------------------------------- host prep
def _host_prep(edge_index, batch):
    src = np.asarray(edge_index[0], dtype=np.int64)
    dst = np.asarray(edge_index[1], dtype=np.int64)
    batch = np.asarray(batch, dtype=np.int64)

    deg = np.bincount(dst, minlength=N).astype(np.int64)          # in-degree
    deg_ext = np.concatenate([deg, np.full(NTOT - N, -1, np.int64)])
    order = np.argsort(-deg_ext, kind="stable")                   # rank -> orig
    rank = np.empty(NTOT, np.int64)
    rank[order] = np.arange(NTOT)

    c_of = rank % NCORES                                          # node -> core
    j_of = rank // NCORES                                         # local index
    p_of = j_of % 128                                             # partition
    bb_of = j_of // 128                                           # block
    grow = c_of * PERC + p_of * NB + bb_of                        # node -> table row
    pidx = (grow // LANES).astype(np.int16)                       # packed row id
    lane = (grow % LANES).astype(np.int64)

    # per-block gather counts (identical across cores by construction)
    Ks = np.zeros(NB, np.int64)
    d_sorted = np.maximum(deg_ext[order], 0)                      # by rank
    blk_of_rank = (np.arange(NTOT) // NCORES) // 128
    np.maximum.at(Ks, blk_of_rank, d_sorted)
    cumK = np.concatenate([[0], np.cumsum(Ks)]).astype(np.int64)
    TOTK = int(cumK[-1])

    # slot tables: per edge e with dst d: slot (p_of[d], k) of block bb_of[d]
    eo = np.argsort(dst, kind="stable")
    sd = dst[eo]
    se = src[eo]
    node_start = np.searchsorted(sd, np.arange(N))
    k_within = np.arange(len(sd)) - node_start[sd]
    c_e, p_e, bb_e = c_of[sd], p_of[sd], bb_of[sd]
    i_e = k_within * 128 + p_e                                    # element index in block
    idxw = np.zeros((NCORES, 16, 8 * TOTK), np.int16)
    idxw[c_e, i_e % 16, 8 * cumK[bb_e] + i_e // 16] = pidx[se]
    maskw = np.zeros((NCORES, 128, LANES * TOTK), np.float32)
    maskw[c_e, p_e, LANES * (cumK[bb_e] + k_within) + lane[se]] = (
        1.0 / np.maximum(deg[sd], 1)
    )

    # pooling tables (batch is sorted; graph g -> core g//GPC, partition g%GPC)
    cnt = np.bincount(batch, minlength=B).astype(np.int64)
    starts = np.concatenate([[0], np.cumsum(cnt)])
    KP = int(cnt.max())
    KP = -(-KP // 3) * 3                                          # pad to 3 chunks
    nn = np.arange(N)
    g_n = batch
    k_n = nn - starts[g_n]
    cp_n, pp_n = g_n // GPC, g_n % GPC
    i_n = k_n * 128 + pp_n
    pidxw = np.zeros((NCORES, 16, 8 * KP), np.int16)
    pidxw[cp_n, i_n % 16, i_n // 16] = pidx[nn]
    pmaskw = np.zeros((NCORES, 128, LANES * KP), np.float32)
    pmaskw[cp_n, pp_n, LANES * k_n + lane[nn]] = 1.0 / np.maximum(cnt[g_n], 1)

    return dict(
        order=order, Ks=[int(v) for v in Ks], cumK=cumK, TOTK=TOTK, KP=KP,
        idxw=idxw, maskw=maskw, pidxw=pidxw, pmaskw=pmaskw,
    )


def _host_inputs(prep, x, W1, W2, Wfc):
    x = np.asarray(x, np.float32)
    W1 = np.asarray(W1, np.float32)
    W2 = np.asarray(W2, np.float32)
    Wfc = np.asarray(Wfc, np.float32)
    x_ext = np.concatenate([x, np.zeros((NTOT - N, F_IN), np.float32)], 0)
    W1cat = np.concatenate([W1[:F_IN], W1[F_IN:]], axis=1)        # [128, 20]
    W2cat = np.zeros((F16, 2 * DIM), np.float32)
    W2cat[:DIM, :DIM] = W2[:DIM]
    W2cat[:DIM, DIM:] = W2[DIM:]
    wfc_t = np.zeros((128, F16), np.float32)
    wfc_t[:, :DIM] = Wfc[:, 0]

    in_maps = []
    order = prep["order"]
    for c in range(NCORES):
        oc = order[c::NCORES]
        in_maps.append({
            "xT": np.ascontiguousarray(x_ext[oc].T),              # [128, 12544]
            "idxw": np.ascontiguousarray(np.tile(prep["idxw"][c], (8, 1))),
            "maskw": np.ascontiguousarray(prep["maskw"][c]),
            "pidxw": np.ascontiguousarray(np.tile(prep["pidxw"][c], (8, 1))),
            "pmaskw": np.ascontiguousarray(prep["pmaskw"][c]),
            "W1cat": W1cat,
            "W2cat": W2cat,
            "wfc": wfc_t,
        })
    return in_maps


# -------------------------------------------------------------- kernel build
def _build_bass(Ks, TOTK, KP):
    import concourse.bass as bass
    import concourse.mybir as mybir
    import concourse.tile as tile
    from concourse import bacc
    from concourse.masks import make_identity

    f32 = mybir.dt.float32
    i16 = mybir.dt.int16
    AF = mybir.ActivationFunctionType
    ALU = mybir.AluOpType
    AX = mybir.AxisListType
    RG = [list(range(NCORES))]
    cumK = np.concatenate([[0], np.cumsum(Ks)]).astype(np.int64)
    EL = LANES * F16                                              # 64 f32 / row

    nc = bacc.Bacc(num_devices=NCORES)

    xT = nc.dram_tensor("xT", [128, PERC], f32, kind="ExternalInput")
    idxw = nc.dram_tensor("idxw", [128, 8 * TOTK], i16, kind="ExternalInput")
    maskw = nc.dram_tensor("maskw", [128, LANES * TOTK], f32, kind="ExternalInput")
    pidxw = nc.dram_tensor("pidxw", [128, 8 * KP], i16, kind="ExternalInput")
    pmaskw = nc.dram_tensor("pmaskw", [128, LANES * KP], f32, kind="ExternalInput")
    W1cat = nc.dram_tensor("W1cat", [128, 2 * DIM], f32, kind="ExternalInput")
    W2cat = nc.dram_tensor("W2cat", [F16, 2 * DIM], f32, kind="ExternalInput")
    wfc = nc.dram_tensor("wfc", [128, F16], f32, kind="ExternalInput")
    out = nc.dram_tensor("out", [128, 1], f32, kind="ExternalOutput")

    ag_in = [nc.dram_tensor(f"ag{i}_in", [PERC, F16], f32, kind="Internal")
             for i in range(3)]
    ag_out = [nc.dram_tensor(f"ag{i}_out", [R4, EL], f32, kind="Internal",
                             addr_space="Shared") for i in range(3)]

    def gather_block(table, msg, idx_ap, K):
        """Gather 128*K slots into msg [128, K*EL] in <=CAP-idx calls."""
        done = 0
        while done < K:
            kc = min(K - done, CAP // 128)
            nc.gpsimd.dma_gather(
                out_ap=msg[:, EL * done:EL * (done + kc)]
                    .rearrange("p (k f) -> p k f", f=EL),
                in_ap=table[:, :],
                idxs_ap=idx_ap[:, 8 * done:8 * (done + kc)],
                num_idxs=128 * kc,
                num_idxs_reg=128 * kc,
                elem_size=EL,
            )
            done += kc

    with tile.TileContext(nc) as tc:
        with (
            tc.tile_pool(name="const", bufs=1) as cpool,
            tc.tile_pool(name="store", bufs=1) as spool,
            tc.tile_pool(name="stream", bufs=2) as ipool,
            tc.tile_pool(name="msg", bufs=3) as mpool,
            tc.tile_pool(name="prod", bufs=2) as qpool,
            tc.tile_pool(name="agg", bufs=3) as wpool,
            tc.tile_pool(name="psum", bufs=4, space="PSUM") as ppool,
        ):
            # ---- constants / persistent inputs
            ident = cpool.tile([128, 128], f32)
            make_identity(nc, ident[:])
            w1_sb = cpool.tile([128, 2 * DIM], f32)
            nc.sync.dma_start(out=w1_sb[:], in_=W1cat[:, :])
            w2_sb = cpool.tile([F16, 2 * DIM], f32)
            nc.sync.dma_start(out=w2_sb[:], in_=W2cat[:, :])
            wfc_sb = cpool.tile([128, F16], f32)
            nc.sync.dma_start(out=wfc_sb[:], in_=wfc[:, :])
            xT_sb = cpool.tile([128, PERC], f32)
            nc.sync.dma_start(out=xT_sb[:], in_=xT[:, :])

            # ---- persistent stores
            s1_all = spool.tile([128, NB * DIM], f32)    # x @ W1_top
            h_all = spool.tile([128, NB * F16], f32)     # relu layer-1 out (padded)
            z_all = spool.tile([128, NB * DIM], f32)     # h @ W2_top
            y1_all = spool.tile([128, NB * F16], f32)
            y2_all = spool.tile([128, NB * F16], f32)
            h2_all = spool.tile([128, NB * F16], f32)
            nc.vector.memset(h_all[:], 0.0)
            nc.vector.memset(y1_all[:], 0.0)
            nc.vector.memset(y2_all[:], 0.0)
            nc.vector.memset(h2_all[:], 0.0)

            # ================= phase A: layer-1 projection =================
            for b in range(NB):
                ps = ppool.tile([128, 2 * DIM], f32, tag="proj")
                nc.tensor.matmul(out=ps[:], lhsT=xT_sb[:, 128 * b:128 * (b + 1)],
                                 rhs=w1_sb[:], start=True, stop=True)
                nc.scalar.activation(out=s1_all[:, DIM * b:DIM * (b + 1)],
                                     in_=ps[:, :DIM], func=AF.Copy)
                nc.vector.tensor_copy(out=y1_all[:, F16 * b:F16 * b + DIM],
                                      in_=ps[:, DIM:])
            # table rows are (p*NB + b): one contiguous DMA run per partition
            nc.sync.dma_start(
                out=ag_in[0][:, :].rearrange("(p b) f -> p b f", p=128),
                in_=y1_all[:].rearrange("p (b f) -> p b f", f=F16))
            nc.gpsimd.collective_compute(
                "AllGather", mybir.AluOpType.bypass, replica_groups=RG,
                ins=[ag_in[0][:, :]], outs=[ag_out[0][:, :]])

            # ================= phases B/D: aggregation =====================
            def aggregate(table, src_store, dst_store, relu):
                for b0 in range(0, NB, SBLK):
                    c0, c1 = 8 * int(cumK[b0]), 8 * int(cumK[b0 + SBLK])
                    m0 = LANES * int(cumK[b0])
                    idx_t = ipool.tile([128, c1 - c0], i16, tag="idx")
                    nc.sync.dma_start(out=idx_t[:], in_=idxw[:, c0:c1])
                    msk_t = ipool.tile([128, (c1 - c0) // 2], f32, tag="msk")
                    nc.sync.dma_start(
                        out=msk_t[:],
                        in_=maskw[:, m0:m0 + (c1 - c0) // 2])
                    for b in range(b0, b0 + SBLK):
                        K = Ks[b]
                        ioff = 8 * int(cumK[b]) - c0
                        moff = LANES * int(cumK[b]) - m0
                        M = LANES * K
                        msg = mpool.tile([128, K * EL], f32, tag="msg")
                        gather_block(table, msg, idx_t[:, ioff:ioff + 8 * K], K)
                        # lane-select + pad-mask + 1/deg in one op (live DIM only)
                        prod = qpool.tile([128, M * DIM], f32, tag="prod")
                        nc.vector.tensor_mul(
                            out=prod[:].rearrange("p (m f) -> p m f", f=DIM),
                            in0=msg[:].rearrange("p (m f) -> p m f",
                                                 f=F16)[:, :, :DIM],
                            in1=msk_t[:, moff:moff + M].unsqueeze(2)
                                .broadcast_to((128, M, DIM)),
                        )
                        agg = wpool.tile([128, DIM], f32, tag="agg")
                        nc.vector.tensor_reduce(
                            out=agg[:],
                            in_=prod[:].rearrange("p (m f) -> p f m", f=DIM),
                            axis=AX.X, op=ALU.add)
                        nc.vector.tensor_add(
                            out=agg[:], in0=agg[:],
                            in1=src_store[:, DIM * b:DIM * b + DIM])
                        nc.scalar.activation(
                            out=dst_store[:, F16 * b:F16 * b + DIM],
                            in_=agg[:],
                            func=AF.Relu if relu else AF.Copy)

            aggregate(ag_out[0], s1_all, h_all, relu=True)

            # ================= phase C: layer-2 projection =================
            for b in range(NB):
                psT = ppool.tile([F16, 128], f32, tag="psT")
                nc.tensor.transpose(out=psT[:], in_=h_all[:, F16 * b:F16 * (b + 1)],
                                    identity=ident[:])
                hT = wpool.tile([F16, 128], f32, tag="hT")
                nc.vector.tensor_copy(out=hT[:], in_=psT[:])
                ps2 = ppool.tile([128, 2 * DIM], f32, tag="proj")
                nc.tensor.matmul(out=ps2[:], lhsT=hT[:], rhs=w2_sb[:],
                                 start=True, stop=True)
                nc.scalar.activation(out=z_all[:, DIM * b:DIM * (b + 1)],
                                     in_=ps2[:, :DIM], func=AF.Copy)
                nc.vector.tensor_copy(out=y2_all[:, F16 * b:F16 * b + DIM],
                                      in_=ps2[:, DIM:])
            nc.sync.dma_start(
                out=ag_in[1][:, :].rearrange("(p b) f -> p b f", p=128),
                in_=y2_all[:].rearrange("p (b f) -> p b f", f=F16))
            nc.gpsimd.collective_compute(
                "AllGather", mybir.AluOpType.bypass, replica_groups=RG,
                ins=[ag_in[1][:, :]], outs=[ag_out[1][:, :]])

            aggregate(ag_out[1], z_all, h2_all, relu=False)
            nc.sync.dma_start(
                out=ag_in[2][:, :].rearrange("(p b) f -> p b f", p=128),
                in_=h2_all[:].rearrange("p (b f) -> p b f", f=F16))
            nc.gpsimd.collective_compute(
                "AllGather", mybir.AluOpType.bypass, replica_groups=RG,
                ins=[ag_in[2][:, :]], outs=[ag_out[2][:, :]])

            # ================= phase E: pooling + FC + sigmoid =============
            KC = KP // 3
            pool10 = wpool.tile([128, DIM], f32, tag="pool")
            nc.vector.memset(pool10[:], 0.0)
            pidx_t = ipool.tile([128, 8 * KP], i16, tag="pidx")
            nc.sync.dma_start(out=pidx_t[:], in_=pidxw[:, :])
            pmsk_t = ipool.tile([128, LANES * KP], f32, tag="pmsk")
            nc.sync.dma_start(out=pmsk_t[:], in_=pmaskw[:, :])
            for ch in range(3):
                M = LANES * KC
                msg = mpool.tile([128, KC * EL], f32, tag="msg")
                gather_block(ag_out[2], msg,
                             pidx_t[:, 8 * KC * ch:8 * KC * (ch + 1)], KC)
                prod = qpool.tile([128, M * DIM], f32, tag="prod")
                nc.vector.tensor_mul(
                    out=prod[:].rearrange("p (m f) -> p m f", f=DIM),
                    in0=msg[:].rearrange("p (m f) -> p m f", f=F16)[:, :, :DIM],
                    in1=pmsk_t[:, M * ch:M * (ch + 1)].unsqueeze(2)
                        .broadcast_to((128, M, DIM)),
                )
                part = wpool.tile([128, DIM], f32, tag="agg")
                nc.vector.tensor_reduce(
                    out=part[:],
                    in_=prod[:].rearrange("p (m f) -> p f m", f=DIM),
                    axis=AX.X, op=ALU.add)
                nc.vector.tensor_add(out=pool10[:], in0=pool10[:], in1=part[:])
            nc.vector.tensor_mul(out=pool10[:], in0=pool10[:], in1=wfc_sb[:, :DIM])
            logit = wpool.tile([128, 1], f32, tag="logit")
            nc.vector.tensor_reduce(out=logit[:], in_=pool10[:],
                                    axis=AX.X, op=ALU.add)
            res = wpool.tile([128, 1], f32, tag="res")
            nc.scalar.activation(out=res[:], in_=logit[:], func=AF.Sigmoid)
            nc.sync.dma_start(out=out[:, :], in_=res[:])

    nc.finalize()
    return nc


# ------------------------------------------------------------------- driver
def kernel(**inputs) -> np.ndarray:
    from concourse.bass_utils import run_bass_kernel_spmd

    edge_index = np.asarray(inputs["edge_index"])
    batch = np.asarray(inputs["batch"])
    key = (edge_index.shape, int(edge_index[:, ::997].sum()), int(batch[::997].sum()))
    if key not in _CACHE:
        prep = _host_prep(edge_index, batch)
        nc = _build_bass(prep["Ks"], prep["TOTK"], prep["KP"])
        _CACHE[key] = (prep, nc)
    prep, nc = _CACHE[key]

    in_maps = _host_inputs(prep, inputs["x"], inputs["W1"], inputs["W2"],
                           inputs["Wfc"])
    res = run_bass_kernel_spmd(nc, in_maps, core_ids=list(range(NCORES)))
    parts = [res.results[c]["out"][:GPC, :] for c in range(NCORES)]
    return np.concatenate(parts, axis=0).astype(np.float32)
